# revision 41
# baseline (speedup 1.0000x reference)
"""ArcFace head on 8 TRN2 NeuronCores (classifier-parallel / Partial-FC).

out = S * clip(normalize(features) @ normalize(weight).T), with the target
column per row replaced by S * cos(acos(clip(c_tgt)) + M).

Sharding: classes (50000) split 6250/core (padded to 6272 = 49*128 with unit
dummy rows); features replicated. Rows are permuted per core so rows whose
label lands in the core's shard come LAST, so the margin update touches only
the final row-tiles, long after the tiny margin path has finished.
No collectives needed.

Layout: the host folds the (0.01% of total FLOPs) L2 normalization and the
scale S into the inputs and casts them to bf16; the device loads both GEMM
operands pre-transposed straight from HBM via DMA XBAR transpose (2-byte
dtype requirement), so the tensor engine runs pure GEMM with zero transpose
passes and HBM read traffic halves. All transposes ride the scalar hwdge
queue (concurrent transposes on sync+scalar wedge the device); output
stores (26 x ~4MB) ride the sync queue; the margin tiny-path keeps the
vector/scalar copy streams clear by running on gpsimd where possible.
Output stays full fp32 (4096 x 50000).

Self-contained: hardcodes shapes, builds + compiles a Bass/Tile kernel at
call time, runs it via run_bass_kernel_spmd on cores 0-7, reassembles the
full (4096, 50000) output on the host (pure indexing only).
"""

import sys

import numpy as np

for _p in ("/opt/trn_rl_repo",):
    if _p not in sys.path:
        sys.path.insert(0, _p)

import ml_dtypes

S = 30.0
MARGIN = 0.3
EPS = 1e-7
CLIP_HI = float(np.float32(1.0 - EPS))
CLIP_LO = float(np.float32(-1.0 + EPS))
COS_M = float(np.cos(np.float32(MARGIN)))
SIN_M = float(np.sin(np.float32(MARGIN)))

B, D, C = 4096, 512, 50000
NCORES = 8
CS = C // NCORES          # 6250 real classes per core
CSP = 6272                # padded shard width (49 * 128 = 12*512 + 128)
NTF = 512                 # psum free-dim tile (one PSUM bank of fp32)
KT = D // 128             # 4 contraction tiles
TILES = [(nt * 512, 512) for nt in range(12)] + [(6144, 128)]
NT = len(TILES)
# narrow tile early so the kernel ends on wide streaming stores
NTORD = [0, 12] + list(range(1, 12))
MT = B // 128             # 32 row tiles
FQ = 4                    # fT loaded as 4 quarter tiles of 1024 rows


def _build(LP):
    """Build the per-core Bass graph; LP = padded hit-row count (mult of 128)."""
    import concourse.bass as bass  # noqa: F401  (import side effects)
    import concourse.tile as tile
    from concourse import bacc, mybir

    f32 = mybir.dt.float32
    bf16 = mybir.dt.bfloat16
    ALU = mybir.AluOpType
    NMT = LP // 128

    nc = bacc.Bacc(
        "TRN2",
        target_bir_lowering=False,
        debug=False,
        enable_asserts=False,
        num_devices=NCORES,
    )

    i32 = mybir.dt.int32
    fhat_in = nc.dram_tensor("fhat", [B, D], bf16, kind="ExternalInput").ap()
    what_in = nc.dram_tensor("what", [CSP, D], bf16, kind="ExternalInput").ap()
    wsel_in = nc.dram_tensor("wsel", [LP, D], bf16, kind="ExternalInput").ap()
    labadj_in = nc.dram_tensor("labadj", [128, NMT * NT], f32, kind="ExternalInput").ap()
    out_d = nc.dram_tensor("out", [B, CSP], f32, kind="ExternalOutput").ap()

    with tile.TileContext(nc) as tc:
        with (
            tc.tile_pool(name="const", bufs=1) as constp,
            tc.tile_pool(name="ftp", bufs=1) as ftp,
            tc.tile_pool(name="wtp", bufs=4) as wtp,
            tc.tile_pool(name="selstage", bufs=2) as selstage,
            tc.tile_pool(name="stagep", bufs=3) as stagep,
            tc.tile_pool(name="stg12p", bufs=2) as stg12p,
            tc.tile_pool(name="updp", bufs=1) as updp,
            tc.tile_pool(name="smalls", bufs=6) as smalls,
            tc.tile_pool(name="psmm", bufs=8, space="PSUM") as psmm,
        ):
            iota_i = constp.tile([128, NTF], i32, name="iota_i")
            nc.gpsimd.iota(iota_i[:], pattern=[[1, NTF]], base=0, channel_multiplier=0)
            iota_sb = constp.tile([128, NTF], f32, name="iota_sb")
            nc.vector.tensor_copy(iota_sb[:], iota_i[:])
            labadj_sb = constp.tile([128, NMT * NT], f32, name="labadj_sb")
            sdelta = constp.tile([128, NMT], f32, name="sdelta")

            # ---- operand loads: XBAR transpose straight from HBM ----
            # fT[q][p, k, m] = fhat[q*1024 + m, k*128 + p]
            fT = [
                ftp.tile([128, KT, B // FQ], bf16, name=f"fT{q}") for q in range(FQ)
            ]

            def w_prep(nt):
                cstart, ncols = TILES[nt]
                wT = wtp.tile([128, KT, ncols], bf16, name="wT", tag="wT")
                nc.scalar.dma_start_transpose(
                    out=wT[:], in_=what_in[cstart:cstart + ncols, :]
                )
                return wT

            # scalar-queue order: first weight tile, then fT0, so the first
            # matmul fires as soon as fT0 lands; the remaining fT quarters
            # next; later weight tiles stream from the loop under matmuls
            wt_pre = {NTORD[0]: w_prep(NTORD[0])}
            nc.scalar.dma_start_transpose(
                out=fT[0][:], in_=fhat_in[0:1024, :]
            )
            nc.scalar.dma_start_transpose(
                out=fT[1][:], in_=fhat_in[1024:2048, :]
            )
            wt_pre[NTORD[1]] = w_prep(NTORD[1])
            for q in range(2, FQ):
                nc.scalar.dma_start_transpose(
                    out=fT[q][:], in_=fhat_in[q * 1024:(q + 1) * 1024, :]
                )

            # ---- tiny path: margin delta per hit row (all on gpsimd, which
            # is otherwise idle, so the vector/scalar copy streams stay
            # clear; the one sqrt runs on scalar after its transposes) ----
            # hit rows live at the END of the permuted batch (rows B-LP..B);
            # wsel rows are pre-normalized, fhat rows carry S, so the cosine
            # is just dot(fhat_row, wsel_row) / S.
            nc.gpsimd.dma_start(out=labadj_sb[:], in_=labadj_in[:, :])
            fs_all = selstage.tile([128, NMT, D], bf16, name="fs_all")
            nc.gpsimd.dma_start(
                out=fs_all[:],
                in_=fhat_in[B - NMT * 128:B, :].rearrange("(s p) d -> p s d", p=128),
            )
            ws_all = selstage.tile([128, NMT, D], bf16, name="ws_all")
            nc.gpsimd.dma_start(
                out=ws_all[:],
                in_=wsel_in[0:NMT * 128, :].rearrange("(s p) d -> p s d", p=128),
            )
            pscr = selstage.tile([128, NMT, D], f32, name="pscr")
            nc.gpsimd.tensor_mul(pscr[:], fs_all[:], ws_all[:])

            def tiny_tail():
                """Rest of the margin-delta chain: one batched vector reduce
                (emitted after the first group's copies so it doesn't block
                the vector copy stream), the rest on gpsimd + one scalar
                sqrt that lands right after scalar's transpose block."""
                sp = smalls.tile([128, NMT], f32, name="sp")
                nc.vector.reduce_sum(sp[:], pscr[:], mybir.AxisListType.X)
                # c = sp / S, then clip
                ct = smalls.tile([128, NMT], f32, name="ct")
                nc.vector.tensor_scalar(
                    out=ct[:], in0=sp[:], scalar1=float(1.0 / S), scalar2=CLIP_HI,
                    op0=ALU.mult, op1=ALU.min,
                )
                ccl = smalls.tile([128, NMT], f32, name="ccl")
                nc.vector.tensor_scalar(
                    out=ccl[:], in0=ct[:], scalar1=CLIP_LO, scalar2=None,
                    op0=ALU.max,
                )
                c2 = smalls.tile([128, NMT], f32, name="c2")
                nc.vector.tensor_mul(c2[:], ccl[:], ccl[:])
                om = smalls.tile([128, NMT], f32, name="om")
                nc.vector.tensor_scalar(
                    out=om[:], in0=c2[:], scalar1=-1.0, scalar2=1.0,
                    op0=ALU.mult, op1=ALU.add,
                )
                t1 = smalls.tile([128, NMT], f32, name="t1")
                nc.vector.tensor_scalar(
                    out=t1[:], in0=ccl[:], scalar1=float(S * (COS_M - 1.0)),
                    scalar2=None, op0=ALU.mult,
                )
                rt = smalls.tile([128, NMT], f32, name="rt")
                nc.scalar.sqrt(rt[:], om[:])
                # sdelta = S*(cosM-1)*c - S*sinM*sqrt(1-c^2)
                nc.vector.scalar_tensor_tensor(
                    out=sdelta[:],
                    in0=rt[:],
                    scalar=float(-S * SIN_M),
                    in1=t1[:],
                    op0=ALU.mult,
                    op1=ALU.add,
                )

            # ---- main loop: column-tile major, streamed wT blocks ----
            STAGE_M = 16
            out_v = out_d.rearrange("(m p) c -> p m c", p=128)
            plans = {nt: [(0, 16), (16, 16)] for nt in NTORD}
            plans[NTORD[-1]] = [(0, 16), (16, 8), (24, 8)]  # short final store
            for nt in NTORD:
                cstart, ncols = TILES[nt]
                wT = wt_pre[nt] if nt in wt_pre else w_prep(nt)

                for m0, mlen in plans[nt]:
                    # narrow tile uses its own small ring so its stores do
                    # not occupy the wide ring's slots
                    if ncols == 128:
                        stg = stg12p.tile(
                            [128, STAGE_M, 128], f32, name="stg12", tag="stg12"
                        )
                    else:
                        stg = stagep.tile(
                            [128, STAGE_M, NTF], f32, name="stg", tag="stg"
                        )
                    for mi in range(mlen):
                        mt = m0 + mi
                        ps = psmm.tile([128, NTF], f32, name="ps", tag="ps")
                        fq, fo = mt // 8, (mt % 8) * 128
                        for k in range(KT):
                            nc.tensor.matmul(
                                ps[:, :ncols],
                                lhsT=fT[fq][:, k, fo:fo + 128],
                                rhs=wT[:, k, :],
                                start=(k == 0),
                                stop=(k == KT - 1),
                            )
                        dstg = stg[:, mi, :ncols]
                        st = mt - (MT - NMT)
                        if st >= 0:
                            upd = updp.tile([128, NTF], f32, name="upd", tag="upd")
                            nc.vector.tensor_scalar(
                                out=upd[:, :ncols],
                                in0=iota_sb[:, :ncols],
                                scalar1=labadj_sb[:, st * NT + nt: st * NT + nt + 1],
                                scalar2=sdelta[:, st:st + 1],
                                op0=ALU.is_equal,
                                op1=ALU.mult,
                            )
                            nc.vector.tensor_add(dstg, ps[:, :ncols], upd[:, :ncols])
                        elif nt in wt_pre or mt % 2 == 1:
                            # first two tiles: vector only (scalar is still
                            # busy issuing its synchronous transposes)
                            nc.vector.tensor_copy(dstg, ps[:, :ncols])
                        else:
                            nc.scalar.copy(dstg, ps[:, :ncols])
                    nc.sync.dma_start(
                        out=out_v[:, m0:m0 + mlen, cstart:cstart + ncols],
                        in_=stg[:, :mlen, :ncols],
                    )
                    if tiny_tail is not None:
                        tiny_tail()
                        tiny_tail = None

    nc.compile()
    return nc


def _make_in_maps(features, labels, weight, n_cores):
    features = np.ascontiguousarray(features, dtype=np.float32)
    weight = np.ascontiguousarray(weight, dtype=np.float32)
    labels_i = np.asarray(labels).astype(np.int64).ravel()

    fhat = features / np.maximum(
        np.sqrt((features * features).sum(1, keepdims=True)), 1e-12
    )
    fhat = (S * fhat).astype(ml_dtypes.bfloat16)
    what = weight / np.maximum(
        np.sqrt((weight * weight).sum(1, keepdims=True)), 1e-12
    )
    what = what.astype(ml_dtypes.bfloat16)

    core_of = labels_i // CS
    hits = [np.where(core_of == i)[0] for i in range(n_cores)]
    cnt_max = max(len(h) for h in hits)
    LP = max(128, ((cnt_max + 127) // 128) * 128)
    NMT = LP // 128

    in_maps, perms = [], []
    for i in range(n_cores):
        hit = hits[i]
        # hit rows at the END of the batch: margin tiles are processed last
        perm = np.concatenate([np.where(core_of != i)[0], hit])
        perms.append(perm)
        pad = LP - len(hit)
        wsh = np.zeros((CSP, D), ml_dtypes.bfloat16)
        wsh[:CS] = what[i * CS:(i + 1) * CS]
        wsh[CS:, 0] = 1.0  # unit dummy rows; their columns are discarded
        wsel = np.zeros((LP, D), ml_dtypes.bfloat16)
        wsel[:pad, 0] = 1.0
        wsel[pad:] = what[labels_i[hit]]
        labadj = np.full((128, NMT * NT), -1.0, np.float32)
        if len(hit):
            lc = (labels_i[hit] - i * CS).astype(np.float32)
            r = pad + np.arange(len(hit))
            p, mt = r % 128, r // 128
            for nt, (cstart, _w) in enumerate(TILES):
                labadj[p, mt * NT + nt] = lc - cstart
        in_maps.append(
            dict(
                fhat=fhat[perm],
                what=wsh,
                wsel=wsel,
                labadj=labadj,
            )
        )
    return in_maps, perms, LP


_NC_CACHE = {}


def _ensure_ntff_hook():
    """The agent image's antenv lacks axon_hooks; synthesize it so
    run_bass_kernel_spmd(trace=True) can NTFF-profile via the axon .so."""
    import types

    if "antenv.axon_hooks" in sys.modules:
        return
    sys.path.insert(0, "/root/.axon_site")
    from trn_agent_boot.trn_boot import _ntff_profile_via_ctypes

    mod = types.ModuleType("antenv.axon_hooks")
    _state = {"h": None}
    mod.set_axon_ntff_profile_hook = lambda h: _state.__setitem__("h", h)
    mod.get_axon_ntff_profile_hook = lambda: _state["h"]
    sys.modules["antenv.axon_hooks"] = mod
    import antenv

    antenv.axon_hooks = mod
    mod.set_axon_ntff_profile_hook(
        _ntff_profile_via_ctypes("/opt/axon/libaxon_pjrt.so")
    )


def run(features, labels, weight, trace=False, matmul_dtype="bfloat16"):
    """Returns (out, BassKernelResults). matmul_dtype is accepted for
    harness compatibility; the kernel always runs bf16 operands."""
    import concourse.bass_utils as bass_utils
    from concourse.bass_utils import run_bass_kernel_spmd

    if trace:
        _ensure_ntff_hook()
        # no S3 in this container; keep artifacts local
        bass_utils.upload_artifacts = lambda tmpdir: tmpdir

    in_maps, perms, LP = _make_in_maps(features, labels, weight, NCORES)
    if LP not in _NC_CACHE:
        _NC_CACHE[LP] = _build(LP)
    nc = _NC_CACHE[LP]
    res = run_bass_kernel_spmd(
        nc, in_maps, core_ids=list(range(NCORES)), trace=trace
    )
    out = np.empty((B, C), np.float32)
    for i in range(NCORES):
        out[perms[i], i * CS:(i + 1) * CS] = res.results[i]["out"][:, :CS]
    return out, res


def kernel(features, labels, weight):
    out, _ = run(features, labels, weight)
    return out


# revision 42
# speedup vs baseline: 1.0203x; 1.0203x over previous
"""ArcFace head on 8 TRN2 NeuronCores (classifier-parallel / Partial-FC).

out = S * clip(normalize(features) @ normalize(weight).T), with the target
column per row replaced by S * cos(acos(clip(c_tgt)) + M).

Sharding: classes (50000) split 6250/core (padded to 6272 = 49*128 with unit
dummy rows); features replicated. Rows are permuted per core so rows whose
label lands in the core's shard come LAST, so the margin update touches only
the final row-tiles, long after the tiny margin path has finished.
No collectives needed.

Layout: the host folds the (0.01% of total FLOPs) L2 normalization and the
scale S into the inputs and casts them to bf16; the device loads both GEMM
operands pre-transposed straight from HBM via DMA XBAR transpose (2-byte
dtype requirement), so the tensor engine runs pure GEMM with zero transpose
passes and HBM read traffic halves. All transposes ride the scalar hwdge
queue (concurrent transposes on sync+scalar wedge the device); output
stores (26 x ~4MB) ride the sync queue; the margin tiny-path keeps the
vector/scalar copy streams clear by running on gpsimd where possible.
Output stays full fp32 (4096 x 50000).

Self-contained: hardcodes shapes, builds + compiles a Bass/Tile kernel at
call time, runs it via run_bass_kernel_spmd on cores 0-7, reassembles the
full (4096, 50000) output on the host (pure indexing only).
"""

import sys

import numpy as np

for _p in ("/opt/trn_rl_repo",):
    if _p not in sys.path:
        sys.path.insert(0, _p)

import ml_dtypes

S = 30.0
MARGIN = 0.3
EPS = 1e-7
CLIP_HI = float(np.float32(1.0 - EPS))
CLIP_LO = float(np.float32(-1.0 + EPS))
COS_M = float(np.cos(np.float32(MARGIN)))
SIN_M = float(np.sin(np.float32(MARGIN)))

B, D, C = 4096, 512, 50000
NCORES = 8
CS = C // NCORES          # 6250 real classes per core
CSP = 6272                # padded shard width (49 * 128 = 12*512 + 128)
NTF = 512                 # psum free-dim tile (one PSUM bank of fp32)
KT = D // 128             # 4 contraction tiles
TILES = [(nt * 512, 512) for nt in range(12)] + [(6144, 128)]
NT = len(TILES)
# narrow tile early so the kernel ends on wide streaming stores
NTORD = [0, 12] + list(range(1, 12))
MT = B // 128             # 32 row tiles
FQ = 4                    # fT loaded as 4 quarter tiles of 1024 rows


def _build(LP):
    """Build the per-core Bass graph; LP = padded hit-row count (mult of 128)."""
    import concourse.bass as bass  # noqa: F401  (import side effects)
    import concourse.tile as tile
    from concourse import bacc, mybir

    f32 = mybir.dt.float32
    bf16 = mybir.dt.bfloat16
    ALU = mybir.AluOpType
    NMT = LP // 128

    nc = bacc.Bacc(
        "TRN2",
        target_bir_lowering=False,
        debug=False,
        enable_asserts=False,
        num_devices=NCORES,
    )

    i32 = mybir.dt.int32
    fhat_in = nc.dram_tensor("fhat", [B, D], bf16, kind="ExternalInput").ap()
    what_in = nc.dram_tensor("what", [CSP, D], bf16, kind="ExternalInput").ap()
    wsel_in = nc.dram_tensor("wsel", [LP, D], bf16, kind="ExternalInput").ap()
    labadj_in = nc.dram_tensor("labadj", [128, NMT * NT], f32, kind="ExternalInput").ap()
    out_d = nc.dram_tensor("out", [B, CSP], f32, kind="ExternalOutput").ap()

    with tile.TileContext(nc) as tc:
        with (
            tc.tile_pool(name="const", bufs=1) as constp,
            tc.tile_pool(name="ftp", bufs=1) as ftp,
            tc.tile_pool(name="wtp", bufs=4) as wtp,
            tc.tile_pool(name="selstage", bufs=2) as selstage,
            tc.tile_pool(name="stagep", bufs=3) as stagep,
            tc.tile_pool(name="updp", bufs=2) as updp,
            tc.tile_pool(name="smalls", bufs=6) as smalls,
            tc.tile_pool(name="psmm", bufs=8, space="PSUM") as psmm,
        ):
            iota_i = constp.tile([128, NTF], i32, name="iota_i")
            nc.gpsimd.iota(iota_i[:], pattern=[[1, NTF]], base=0, channel_multiplier=0)
            iota_sb = constp.tile([128, NTF], f32, name="iota_sb")
            nc.vector.tensor_copy(iota_sb[:], iota_i[:])
            labadj_sb = constp.tile([128, NMT * NT], f32, name="labadj_sb")
            sdelta = constp.tile([128, NMT], f32, name="sdelta")

            # ---- operand loads: XBAR transpose straight from HBM ----
            # fT[q][p, k, m] = fhat[q*1024 + m, k*128 + p]
            fT = [
                ftp.tile([128, KT, B // FQ], bf16, name=f"fT{q}") for q in range(FQ)
            ]

            def w_prep(nt):
                cstart, ncols = TILES[nt]
                wT = wtp.tile([128, KT, ncols], bf16, name="wT", tag="wT")
                nc.scalar.dma_start_transpose(
                    out=wT[:], in_=what_in[cstart:cstart + ncols, :]
                )
                return wT

            # scalar-queue order: first weight tile, then fT0, so the first
            # matmul fires as soon as fT0 lands; the remaining fT quarters
            # next; later weight tiles stream from the loop under matmuls
            wt_pre = {NTORD[0]: w_prep(NTORD[0])}
            nc.scalar.dma_start_transpose(
                out=fT[0][:], in_=fhat_in[0:1024, :]
            )
            wt_pre[NTORD[1]] = w_prep(NTORD[1])
            for q in range(1, FQ):
                nc.scalar.dma_start_transpose(
                    out=fT[q][:], in_=fhat_in[q * 1024:(q + 1) * 1024, :]
                )

            # ---- tiny path: margin delta per hit row (all on gpsimd, which
            # is otherwise idle, so the vector/scalar copy streams stay
            # clear; the one sqrt runs on scalar after its transposes) ----
            # hit rows live at the END of the permuted batch (rows B-LP..B);
            # wsel rows are pre-normalized, fhat rows carry S, so the cosine
            # is just dot(fhat_row, wsel_row) / S.
            nc.gpsimd.dma_start(out=labadj_sb[:], in_=labadj_in[:, :])
            fs_all = selstage.tile([128, NMT, D], bf16, name="fs_all")
            nc.gpsimd.dma_start(
                out=fs_all[:],
                in_=fhat_in[B - NMT * 128:B, :].rearrange("(s p) d -> p s d", p=128),
            )
            ws_all = selstage.tile([128, NMT, D], bf16, name="ws_all")
            nc.gpsimd.dma_start(
                out=ws_all[:],
                in_=wsel_in[0:NMT * 128, :].rearrange("(s p) d -> p s d", p=128),
            )
            pscr = selstage.tile([128, NMT, D], f32, name="pscr")
            nc.gpsimd.tensor_mul(pscr[:], fs_all[:], ws_all[:])

            def tiny_tail():
                """Rest of the margin-delta chain: one batched vector reduce
                (emitted after the first group's copies so it doesn't block
                the vector copy stream), the rest on gpsimd + one scalar
                sqrt that lands right after scalar's transpose block."""
                sp = smalls.tile([128, NMT], f32, name="sp")
                nc.vector.reduce_sum(sp[:], pscr[:], mybir.AxisListType.X)
                # c = sp / S, then clip
                ct = smalls.tile([128, NMT], f32, name="ct")
                nc.vector.tensor_scalar(
                    out=ct[:], in0=sp[:], scalar1=float(1.0 / S), scalar2=CLIP_HI,
                    op0=ALU.mult, op1=ALU.min,
                )
                ccl = smalls.tile([128, NMT], f32, name="ccl")
                nc.vector.tensor_scalar(
                    out=ccl[:], in0=ct[:], scalar1=CLIP_LO, scalar2=None,
                    op0=ALU.max,
                )
                c2 = smalls.tile([128, NMT], f32, name="c2")
                nc.vector.tensor_mul(c2[:], ccl[:], ccl[:])
                om = smalls.tile([128, NMT], f32, name="om")
                nc.vector.tensor_scalar(
                    out=om[:], in0=c2[:], scalar1=-1.0, scalar2=1.0,
                    op0=ALU.mult, op1=ALU.add,
                )
                t1 = smalls.tile([128, NMT], f32, name="t1")
                nc.vector.tensor_scalar(
                    out=t1[:], in0=ccl[:], scalar1=float(S * (COS_M - 1.0)),
                    scalar2=None, op0=ALU.mult,
                )
                rt = smalls.tile([128, NMT], f32, name="rt")
                nc.scalar.sqrt(rt[:], om[:])
                # sdelta = S*(cosM-1)*c - S*sinM*sqrt(1-c^2)
                nc.vector.scalar_tensor_tensor(
                    out=sdelta[:],
                    in0=rt[:],
                    scalar=float(-S * SIN_M),
                    in1=t1[:],
                    op0=ALU.mult,
                    op1=ALU.add,
                )

            # ---- main loop: column-tile major, streamed wT blocks ----
            STAGE_M = 16
            out_v = out_d.rearrange("(m p) c -> p m c", p=128)
            plans = {nt: [(0, 16), (16, 16)] for nt in NTORD}
            plans[NTORD[-1]] = [(0, 16), (16, 8), (24, 8)]  # short final store
            for nt in NTORD:
                cstart, ncols = TILES[nt]
                wT = wt_pre[nt] if nt in wt_pre else w_prep(nt)

                for m0, mlen in plans[nt]:
                    stg = stagep.tile(
                        [128, STAGE_M, NTF], f32, name="stg", tag="stg"
                    )
                    for mi in range(mlen):
                        mt = m0 + mi
                        ps = psmm.tile([128, NTF], f32, name="ps", tag="ps")
                        fq, fo = mt // 8, (mt % 8) * 128
                        for k in range(KT):
                            nc.tensor.matmul(
                                ps[:, :ncols],
                                lhsT=fT[fq][:, k, fo:fo + 128],
                                rhs=wT[:, k, :],
                                start=(k == 0),
                                stop=(k == KT - 1),
                            )
                        dstg = stg[:, mi, :ncols]
                        st = mt - (MT - NMT)
                        if st >= 0:
                            upd = updp.tile([128, NTF], f32, name="upd", tag="upd")
                            nc.vector.tensor_scalar(
                                out=upd[:, :ncols],
                                in0=iota_sb[:, :ncols],
                                scalar1=labadj_sb[:, st * NT + nt: st * NT + nt + 1],
                                scalar2=sdelta[:, st:st + 1],
                                op0=ALU.is_equal,
                                op1=ALU.mult,
                            )
                            nc.vector.tensor_add(dstg, ps[:, :ncols], upd[:, :ncols])
                        elif nt in wt_pre or mt % 2 == 1:
                            # first two tiles: vector only (scalar is still
                            # busy issuing its synchronous transposes)
                            nc.vector.tensor_copy(dstg, ps[:, :ncols])
                        else:
                            nc.scalar.copy(dstg, ps[:, :ncols])
                    nc.sync.dma_start(
                        out=out_v[:, m0:m0 + mlen, cstart:cstart + ncols],
                        in_=stg[:, :mlen, :ncols],
                    )
                    if tiny_tail is not None:
                        tiny_tail()
                        tiny_tail = None

    nc.compile()
    return nc


def _make_in_maps(features, labels, weight, n_cores):
    features = np.ascontiguousarray(features, dtype=np.float32)
    weight = np.ascontiguousarray(weight, dtype=np.float32)
    labels_i = np.asarray(labels).astype(np.int64).ravel()

    fhat = features / np.maximum(
        np.sqrt((features * features).sum(1, keepdims=True)), 1e-12
    )
    fhat = (S * fhat).astype(ml_dtypes.bfloat16)
    what = weight / np.maximum(
        np.sqrt((weight * weight).sum(1, keepdims=True)), 1e-12
    )
    what = what.astype(ml_dtypes.bfloat16)

    core_of = labels_i // CS
    hits = [np.where(core_of == i)[0] for i in range(n_cores)]
    cnt_max = max(len(h) for h in hits)
    LP = max(128, ((cnt_max + 127) // 128) * 128)
    NMT = LP // 128

    in_maps, perms = [], []
    for i in range(n_cores):
        hit = hits[i]
        # hit rows at the END of the batch: margin tiles are processed last
        perm = np.concatenate([np.where(core_of != i)[0], hit])
        perms.append(perm)
        pad = LP - len(hit)
        wsh = np.zeros((CSP, D), ml_dtypes.bfloat16)
        wsh[:CS] = what[i * CS:(i + 1) * CS]
        wsh[CS:, 0] = 1.0  # unit dummy rows; their columns are discarded
        wsel = np.zeros((LP, D), ml_dtypes.bfloat16)
        wsel[:pad, 0] = 1.0
        wsel[pad:] = what[labels_i[hit]]
        labadj = np.full((128, NMT * NT), -1.0, np.float32)
        if len(hit):
            lc = (labels_i[hit] - i * CS).astype(np.float32)
            r = pad + np.arange(len(hit))
            p, mt = r % 128, r // 128
            for nt, (cstart, _w) in enumerate(TILES):
                labadj[p, mt * NT + nt] = lc - cstart
        in_maps.append(
            dict(
                fhat=fhat[perm],
                what=wsh,
                wsel=wsel,
                labadj=labadj,
            )
        )
    return in_maps, perms, LP


_NC_CACHE = {}


def _ensure_ntff_hook():
    """The agent image's antenv lacks axon_hooks; synthesize it so
    run_bass_kernel_spmd(trace=True) can NTFF-profile via the axon .so."""
    import types

    if "antenv.axon_hooks" in sys.modules:
        return
    sys.path.insert(0, "/root/.axon_site")
    from trn_agent_boot.trn_boot import _ntff_profile_via_ctypes

    mod = types.ModuleType("antenv.axon_hooks")
    _state = {"h": None}
    mod.set_axon_ntff_profile_hook = lambda h: _state.__setitem__("h", h)
    mod.get_axon_ntff_profile_hook = lambda: _state["h"]
    sys.modules["antenv.axon_hooks"] = mod
    import antenv

    antenv.axon_hooks = mod
    mod.set_axon_ntff_profile_hook(
        _ntff_profile_via_ctypes("/opt/axon/libaxon_pjrt.so")
    )


def run(features, labels, weight, trace=False, matmul_dtype="bfloat16"):
    """Returns (out, BassKernelResults). matmul_dtype is accepted for
    harness compatibility; the kernel always runs bf16 operands."""
    import concourse.bass_utils as bass_utils
    from concourse.bass_utils import run_bass_kernel_spmd

    if trace:
        _ensure_ntff_hook()
        # no S3 in this container; keep artifacts local
        bass_utils.upload_artifacts = lambda tmpdir: tmpdir

    in_maps, perms, LP = _make_in_maps(features, labels, weight, NCORES)
    if LP not in _NC_CACHE:
        _NC_CACHE[LP] = _build(LP)
    nc = _NC_CACHE[LP]
    res = run_bass_kernel_spmd(
        nc, in_maps, core_ids=list(range(NCORES)), trace=trace
    )
    out = np.empty((B, C), np.float32)
    for i in range(NCORES):
        out[perms[i], i * CS:(i + 1) * CS] = res.results[i]["out"][:, :CS]
    return out, res


def kernel(features, labels, weight):
    out, _ = run(features, labels, weight)
    return out
